# revision 1
# baseline (speedup 1.0000x reference)
"""Trainium2 Bass kernel for nn_CrossCorrelation_38405597561347.

Reference computation (per sample b, scale l):
  C_l = valid_corr2d(s_l[b], flip2(t_l[b]))          # [L, L], L = H - k + 1
  out[b, :, :, l] = bilinear_resize(C_l, 128, 128)   # jax antialiased triangle
Output: [64, 128, 128, 3] float32.

Mapping to hardware (per core, 8 samples — pure batch data parallelism):
  * Correlation via banded-Toeplitz matmuls: contraction over input rows r,
    accumulated over template columns v in PSUM:
       C[i, j] = sum_v sum_r k[r - i, v] * s[r, j + v]
    lhsT = host-built Toeplitz weight tile W_v[p, q] = k[p - q, v] (plus a
    small second chunk for the 16-row halo), rhs = image rows in SBUF with the
    v-shift expressed as a free-dim AP offset.  float32r (full PE rate).
  * Resize = R @ C @ R^T with the (constant) antialiased triangle matrix R:
    stage 1 contracts C's row blocks (lhsT = R^T chunks), PE-transpose of the
    intermediate, stage 2 contracts the column dim.  Output is [x, y]
    (transposed); host fixes layout during the final gather.
"""

import os
import sys
from dataclasses import dataclass

import ml_dtypes
import numpy as np

for _p in ("/opt/trn_rl_repo",):
    if _p not in sys.path and os.path.isdir(_p):
        sys.path.insert(0, _p)

import concourse.bass as bass
import concourse.mybir as mybir
from concourse.bass_utils import run_bass_kernel_spmd
from concourse.masks import make_identity
from concourse.tile import TileContext

NCORES = 8
B_TOTAL = 64
PER = B_TOTAL // NCORES  # 8 samples per core
OUT_HW = 128

F32 = mybir.dt.float32
F32R = mybir.dt.float32r
BF16 = mybir.dt.bfloat16
# matmul operand dtype: bfloat16 (fast + compiles clean, ~4e-3 rel err).
# float32r is kept switchable for experiments but trips walrus ISA checks.
MM_DT = F32R if os.environ.get("KERNEL_MM_DT", "bf16") == "f32r" else BF16


@dataclass(frozen=True)
class Scale:
    l: int    # scale index
    H: int    # input image height/width
    K: int    # template size
    L: int    # valid correlation output size = H - K + 1
    BS: int   # valid output rows per 128-row image tile (= 128 - K + 1)
    NT: int   # number of overlapping image row-tiles (= ceil(L / BS))
    NPAD: int  # padded correlation free dim
    IMW: int  # per-row-tile SBUF width (>= NPAD + K - 1)
    NJC: int  # number of 128-col chunks fed to transpose/stage2


SCALES = (
    Scale(l=0, H=512, K=16, L=497, BS=113, NT=5, NPAD=497, IMW=512, NJC=4),
    Scale(l=1, H=256, K=8, L=249, BS=121, NT=3, NPAD=256, IMW=264, NJC=2),
    Scale(l=2, H=128, K=4, L=125, BS=125, NT=1, NPAD=256, IMW=264, NJC=1),
)

last_exec_time_ns = None
last_mean_exec_time_ns = None


# ----------------------------------------------------------------- host math
def _resize_mat(lin: int, lout: int = OUT_HW) -> np.ndarray:
    """R[y, i]: jax.image.resize bilinear (antialias=True) weight matrix."""
    inv_scale = lin / lout
    kernel_scale = max(inv_scale, 1.0)
    sample_f = (np.arange(lout, dtype=np.float64) + 0.5) * inv_scale - 0.5
    x = np.abs(sample_f[None, :] - np.arange(lin, dtype=np.float64)[:, None])
    w = np.maximum(0.0, 1.0 - x / kernel_scale)  # [lin, lout]
    w = w / w.sum(axis=0, keepdims=True)
    return np.ascontiguousarray(w.T.astype(np.float32))  # [lout, lin]


def _rta_chunks(sc: Scale) -> np.ndarray:
    """Stage-1 R^T chunks over overlapping BS-stride C tiles.

    [128, NT, 128]: rta[q, t, y] = R[y, BS*t + q] for q < BS (else 0)."""
    r = _resize_mat(sc.L)  # [128, L]
    rt = np.zeros((128, sc.NT, OUT_HW), np.float32)
    for t in range(sc.NT):
        rows = min(sc.BS, sc.L - sc.BS * t)
        if rows > 0:
            rt[:rows, t, :] = r[:, sc.BS * t : sc.BS * t + rows].T
    return rt


def _rtb_chunks(sc: Scale) -> np.ndarray:
    """Stage-2 R^T chunks over 128-aligned D^T tiles.

    [128, NJC, 128]: rtb[p, n, y] = R[y, 128n + p], zero beyond L."""
    r = _resize_mat(sc.L)  # [128, L]
    rt = np.zeros((128, sc.NJC, OUT_HW), np.float32)
    for n in range(sc.NJC):
        rows = min(128, sc.L - 128 * n)
        if rows > 0:
            rt[:rows, n, :] = r[:, 128 * n : 128 * n + rows].T
    return rt


def _tile_image(img: np.ndarray, sc: Scale) -> np.ndarray:
    """[B, H, W] -> [B, NT, 128, IMW] overlapping row tiles at stride BS,
    zero-padded beyond the valid rows/cols."""
    nb = img.shape[0]
    out = np.zeros((nb, sc.NT, 128, sc.IMW), np.float32)
    for t in range(sc.NT):
        r0 = sc.BS * t
        rows = min(128, sc.H - r0)
        out[:, t, :rows, : sc.H] = img[:, r0 : r0 + rows, :]
    return out


def _toeplitz_a(k: np.ndarray, kap: int) -> np.ndarray:
    """k: [n, kap, kap] -> wa[n, p, v, q] = k[n, p-q, v] for 0 <= p-q < kap."""
    p = np.arange(128)[:, None]
    q = np.arange(128)[None, :]
    d = p - q
    m = (d >= 0) & (d < kap)
    dc = np.where(m, d, 0)
    w = k[:, dc, :]  # [n, 128(p), 128(q), kap(v)]
    w = w.transpose(0, 1, 3, 2) * m[None, :, None, :]
    return np.ascontiguousarray(w.astype(np.float32))


# -------------------------------------------------------------- device build
_NC_CACHE: dict = {}

_MAX_WAITS = 1  # walrus setupSyncWait rejects multiple sem waits on some opcodes


def _split_excess_waits(nc, max_waits: int = _MAX_WAITS):
    """Hoist excess semaphore waits onto injected same-engine NoOps.

    Tile's vector-clock wait emission is not transitively minimal, so some
    instructions end up with 3+ sem waits, which walrus codegen rejects.
    Executing the same waits on an immediately preceding NoOp of the same
    engine is semantically identical (engines execute in program order).
    """
    n_id = 0
    for f in nc.m.functions:
        for bb in f.blocks:
            out = []
            changed = False
            for inst in bb.instructions:
                si = inst.sync_info
                waits = list(si.on_wait or []) if si is not None else []
                if len(waits) > max_waits:
                    keep = waits[-max_waits:]
                    excess = waits[:-max_waits]
                    for i in range(0, len(excess), max_waits):
                        n_id += 1
                        out.append(
                            mybir.InstNoOp(
                                name=f"waitnop-{n_id}",
                                engine=inst.engine,
                                bass_nofuse=True,
                                sync_info=mybir.SyncInfo(
                                    on_wait=excess[i : i + max_waits], on_update=[]
                                ),
                            )
                        )
                    inst.sync_info = mybir.SyncInfo(
                        on_wait=keep, on_update=list(si.on_update or [])
                    )
                    changed = True
                out.append(inst)
            if changed:
                bb.instructions = out


def _emit_sample(nc, sc: Scale, b: int, pools, params, consts):
    imgp, wtsp, cbufp, dbufp, obufp, ps_c, ps_d, ps_t, ps_o = pools
    img_p, wa_p, out_p = params
    ident, rta_t, rtb_t = consts

    img_t = imgp.tile([128, sc.NT, sc.IMW], MM_DT, tag=f"img{sc.l}", name=f"img{sc.l}_{b}")
    nc.sync.dma_start(
        out=img_t[:, :, :],
        in_=img_p[b].rearrange("t p c -> p t c"),
    )

    wa_t = wtsp.tile([128, sc.K, 128], MM_DT, tag=f"wa{sc.l}", name=f"wa{sc.l}_{b}")
    nc.sync.dma_start(out=wa_t[:, :, :], in_=wa_p[b])

    # --- correlation + resize stage 1 (row contraction) ---
    d_ps = ps_d.tile([128, sc.NPAD], F32, tag="d", name=f"d_ps{sc.l}_{b}")
    for t in range(sc.NT):
        c_ps = ps_c.tile([128, sc.NPAD], F32, tag="c", name=f"c_ps{sc.l}_{b}_{t}")
        for v in range(sc.K):
            nc.tensor.matmul(
                c_ps[:, :],
                wa_t[:, v, :],
                img_t[:, t, v : v + sc.NPAD],
                start=(v == 0),
                stop=(v == sc.K - 1),
            )
        c_t = cbufp.tile([128, sc.NPAD], MM_DT, tag=f"c{sc.l}", name=f"c{sc.l}_{b}_{t}")
        nc.vector.tensor_copy(c_t[:, :], c_ps[:, :])
        nc.tensor.matmul(
            d_ps[:, :],
            rta_t[:, t, :],
            c_t[:, :],
            start=(t == 0),
            stop=(t == sc.NT - 1),
        )

    dw = sc.NJC * 128  # D buffer width: full transpose chunks, f32-padded tail
    cw = min(sc.NPAD, dw)
    d_t = dbufp.tile([128, dw], F32, tag=f"d{sc.l}", name=f"d{sc.l}_{b}")
    nc.vector.tensor_copy(d_t[:, 0:cw], d_ps[:, 0:cw])
    if dw > sc.NPAD:
        nc.vector.memset(d_t[:, sc.NPAD : dw], 0.0)

    # --- transpose D, then resize stage 2 (col contraction) ---
    dt_t = dbufp.tile([128, sc.NJC, 128], MM_DT, tag=f"dt{sc.l}", name=f"dt{sc.l}_{b}")
    for jc in range(sc.NJC):
        tr_ps = ps_t.tile([128, 128], F32, tag="tr", name=f"tr_ps{sc.l}_{b}_{jc}")
        nc.tensor.transpose(tr_ps[:, :], d_t[:, 128 * jc : 128 * jc + 128], ident)
        nc.vector.tensor_copy(dt_t[:, jc, :], tr_ps[:, :])

    o_ps = ps_o.tile([128, 128], F32, tag="o", name=f"o_ps{sc.l}_{b}")
    for jc in range(sc.NJC):
        nc.tensor.matmul(
            o_ps[:, :],
            rtb_t[:, jc, :],
            dt_t[:, jc, :],
            start=(jc == 0),
            stop=(jc == sc.NJC - 1),
        )
    o_t = obufp.tile([128, 128], F32, tag="o", name=f"o{sc.l}_{b}")
    nc.vector.tensor_copy(o_t[:, :], o_ps[:, :])
    nc.sync.dma_start(out=out_p[b], in_=o_t[:, :])


def _build_nc(split_waits: bool = True):
    key = ("nc", split_waits)
    if key in _NC_CACHE:
        return _NC_CACHE[key]

    nc = bass.Bass()
    img_p, wa_p, rta_p, rtb_p, out_p = [], [], [], [], []
    for sc in SCALES:
        img_p.append(
            nc.declare_dram_parameter(
                f"img{sc.l}", [PER, sc.NT, 128, sc.IMW], MM_DT, isOutput=False
            )
        )
        wa_p.append(
            nc.declare_dram_parameter(f"wa{sc.l}", [PER, 128, sc.K, 128], MM_DT, isOutput=False)
        )
        rta_p.append(
            nc.declare_dram_parameter(f"rta{sc.l}", [128, sc.NT, 128], MM_DT, isOutput=False)
        )
        rtb_p.append(
            nc.declare_dram_parameter(f"rtb{sc.l}", [128, sc.NJC, 128], MM_DT, isOutput=False)
        )
        out_p.append(
            nc.declare_dram_parameter(f"out{sc.l}", [PER, 128, 128], F32, isOutput=True)
        )

    with TileContext(nc) as tc:
        with (
            tc.tile_pool(name="const", bufs=1) as constp,
            tc.tile_pool(name="img", bufs=2) as imgp,
            tc.tile_pool(name="wts", bufs=2) as wtsp,
            tc.tile_pool(name="cbuf", bufs=2) as cbufp,
            tc.tile_pool(name="dbuf", bufs=2) as dbufp,
            tc.tile_pool(name="obuf", bufs=2) as obufp,
            tc.tile_pool(name="ps_c", bufs=2, space="PSUM") as ps_c,
            tc.tile_pool(name="ps_d", bufs=2, space="PSUM") as ps_d,
            tc.tile_pool(name="ps_t", bufs=2, space="PSUM") as ps_t,
            tc.tile_pool(name="ps_o", bufs=2, space="PSUM") as ps_o,
        ):
            ident = constp.tile([128, 128], F32, name="ident")
            make_identity(nc, ident)
            rta_ts, rtb_ts = [], []
            for sc in SCALES:
                rta_t = constp.tile([128, sc.NT, 128], MM_DT, name=f"rta{sc.l}_t")
                nc.sync.dma_start(out=rta_t[:, :, :], in_=rta_p[sc.l][:, :, :])
                rta_ts.append(rta_t)
                rtb_t = constp.tile([128, sc.NJC, 128], MM_DT, name=f"rtb{sc.l}_t")
                nc.sync.dma_start(out=rtb_t[:, :, :], in_=rtb_p[sc.l][:, :, :])
                rtb_ts.append(rtb_t)

            pools = (imgp, wtsp, cbufp, dbufp, obufp, ps_c, ps_d, ps_t, ps_o)
            for b in range(PER):
                for sc in SCALES:
                    _emit_sample(
                        nc,
                        sc,
                        b,
                        pools,
                        (img_p[sc.l], wa_p[sc.l], out_p[sc.l]),
                        (ident, rta_ts[sc.l], rtb_ts[sc.l]),
                    )

    if split_waits:
        _split_excess_waits(nc)
    _NC_CACHE[key] = nc
    return nc


# --------------------------------------------------------------------- entry
def kernel(s1, s2, s3, t1, t2, t3):
    global last_exec_time_ns, last_mean_exec_time_ns

    host_dt = ml_dtypes.bfloat16 if MM_DT == BF16 else np.float32
    imgs = []
    for s, sc in zip((s1, s2, s3), SCALES):
        a = np.asarray(s, np.float32)[..., 0]
        imgs.append(np.ascontiguousarray(_tile_image(a, sc).astype(host_dt)))
    tmps = [np.asarray(t, np.float32)[..., 0] for t in (t1, t2, t3)]
    # flip both spatial dims (reference flips the template before the conv)
    ks = [np.ascontiguousarray(t[:, ::-1, ::-1]) for t in tmps]

    was = [_toeplitz_a(ks[sc.l], sc.K).astype(host_dt) for sc in SCALES]
    rtas = [_rta_chunks(sc).astype(host_dt) for sc in SCALES]
    rtbs = [_rtb_chunks(sc).astype(host_dt) for sc in SCALES]

    nc = _build_nc()

    in_maps = []
    for c in range(NCORES):
        sl = slice(PER * c, PER * (c + 1))
        m = {}
        for sc in SCALES:
            m[f"img{sc.l}"] = imgs[sc.l][sl]
            m[f"wa{sc.l}"] = was[sc.l][sl]
            m[f"rta{sc.l}"] = rtas[sc.l]
            m[f"rtb{sc.l}"] = rtbs[sc.l]
        in_maps.append(m)

    trace = bool(os.environ.get("KERNEL_TRACE"))
    tmpdir = os.environ.get("KERNEL_TRACE_DIR") or None
    if tmpdir:
        os.makedirs(tmpdir, exist_ok=True)
    res = run_bass_kernel_spmd(
        nc, in_maps, list(range(NCORES)), trace=trace, tmpdir=tmpdir
    )
    last_exec_time_ns = res.exec_time_ns
    last_mean_exec_time_ns = res.mean_exec_time_ns

    out = np.empty((B_TOTAL, OUT_HW, OUT_HW, 3), np.float32)
    for c in range(NCORES):
        sl = slice(PER * c, PER * (c + 1))
        for sc in SCALES:
            # device output is [b, x, y]; reference wants [b, y, x]
            out[sl, :, :, sc.l] = res.results[c][f"out{sc.l}"].swapaxes(1, 2)
    return out



# revision 15
# speedup vs baseline: 1.2984x; 1.2984x over previous
"""Trainium2 Bass kernel for nn_CrossCorrelation_38405597561347.

Reference computation (per sample b, scale l):
  C_l = valid_corr2d(s_l[b], flip2(t_l[b]))          # [L, L], L = H - k + 1
  out[b, :, :, l] = bilinear_resize(C_l, 128, 128)   # jax antialiased triangle
Output: [64, 128, 128, 3] float32.

Mapping to hardware (per core, 8 samples — pure batch data parallelism):
  * Correlation via banded-Toeplitz matmuls: contraction over input rows r,
    accumulated over template columns v in PSUM:
       C[i, j] = sum_v sum_r k[r - i, v] * s[r, j + v]
    lhsT = host-built Toeplitz weight tile W_v[p, q] = k[p - q, v] (plus a
    small second chunk for the 16-row halo), rhs = image rows in SBUF with the
    v-shift expressed as a free-dim AP offset.  float32r (full PE rate).
  * Resize = R @ C @ R^T with the (constant) antialiased triangle matrix R:
    stage 1 contracts C's row blocks (lhsT = R^T chunks), PE-transpose of the
    intermediate, stage 2 contracts the column dim.  Output is [x, y]
    (transposed); host fixes layout during the final gather.
"""

import os
import sys
from dataclasses import dataclass

import ml_dtypes
import numpy as np

for _p in ("/opt/trn_rl_repo",):
    if _p not in sys.path and os.path.isdir(_p):
        sys.path.insert(0, _p)

import concourse.bass as bass
import concourse.mybir as mybir
from concourse.bass import AP
from concourse.bass_utils import run_bass_kernel_spmd
from concourse.masks import make_identity
from concourse.tile import TileContext

DOUBLE_ROW = mybir.MatmulPerfMode.DoubleRow
NQ8 = 12  # scale-0 template columns computed in fp8 (rest exact in bf16)

NCORES = 8
B_TOTAL = 64
PER = B_TOTAL // NCORES  # 8 samples per core
OUT_HW = 128

F32 = mybir.dt.float32
F32R = mybir.dt.float32r
BF16 = mybir.dt.bfloat16
F8E4 = mybir.dt.float8e4
# matmul operand dtype: bfloat16 (fast + compiles clean, ~4e-3 rel err).
# float32r is kept switchable for experiments but trips walrus ISA checks.
MM_DT = F32R if os.environ.get("KERNEL_MM_DT", "bf16") == "f32r" else BF16
E4 = ml_dtypes.float8_e4m3


@dataclass(frozen=True)
class Scale:
    l: int    # scale index
    H: int    # input image height/width
    K: int    # template size
    L: int    # valid correlation output size = H - K + 1
    BS: int   # valid output rows per 128-row image tile (= 128 - K + 1)
    NT: int   # number of overlapping image row-tiles (= ceil(L / BS))
    NPAD: int  # padded correlation free dim
    IMW: int  # per-row-tile SBUF width (>= NPAD + K - 1)
    NJC: int  # number of 128-col chunks fed to transpose/stage2
    q8: bool  # correlation operands in fp8e4 + DoubleRow (2x PE rate)


SCALES = (
    Scale(l=0, H=512, K=16, L=497, BS=113, NT=5, NPAD=497, IMW=512, NJC=4, q8=True),
    Scale(l=1, H=256, K=8, L=249, BS=121, NT=3, NPAD=256, IMW=264, NJC=2, q8=False),
    Scale(l=2, H=128, K=4, L=125, BS=125, NT=1, NPAD=128, IMW=136, NJC=1, q8=False),
)

last_exec_time_ns = None
last_mean_exec_time_ns = None


# ----------------------------------------------------------------- host math
def _resize_mat(lin: int, lout: int = OUT_HW) -> np.ndarray:
    """R[y, i]: jax.image.resize bilinear (antialias=True) weight matrix."""
    inv_scale = lin / lout
    kernel_scale = max(inv_scale, 1.0)
    sample_f = (np.arange(lout, dtype=np.float64) + 0.5) * inv_scale - 0.5
    x = np.abs(sample_f[None, :] - np.arange(lin, dtype=np.float64)[:, None])
    w = np.maximum(0.0, 1.0 - x / kernel_scale)  # [lin, lout]
    w = w / w.sum(axis=0, keepdims=True)
    return np.ascontiguousarray(w.T.astype(np.float32))  # [lout, lin]


def _rta_chunks(sc: Scale) -> np.ndarray:
    """Stage-1 R^T chunks over overlapping BS-stride C tiles.

    [128, NT, 128]: rta[q, t, y] = R[y, BS*t + q] for q < BS (else 0)."""
    r = _resize_mat(sc.L)  # [128, L]
    rt = np.zeros((128, sc.NT, OUT_HW), np.float32)
    for t in range(sc.NT):
        rows = min(sc.BS, sc.L - sc.BS * t)
        if rows > 0:
            rt[:rows, t, :] = r[:, sc.BS * t : sc.BS * t + rows].T
    return rt


def _rtb_chunks(sc: Scale) -> np.ndarray:
    """Stage-2 R^T chunks over 128-aligned D^T tiles.

    [128, NJC, 128]: rtb[p, n, y] = R[y, 128n + p], zero beyond L."""
    r = _resize_mat(sc.L)  # [128, L]
    rt = np.zeros((128, sc.NJC, OUT_HW), np.float32)
    for n in range(sc.NJC):
        rows = min(128, sc.L - 128 * n)
        if rows > 0:
            rt[:rows, n, :] = r[:, 128 * n : 128 * n + rows].T
    return rt


def _tile_image(img: np.ndarray, sc: Scale) -> np.ndarray:
    """[B, H, W] -> [B, NT, 128, IMW] overlapping row tiles at stride BS,
    zero-padded beyond the valid rows/cols."""
    nb = img.shape[0]
    out = np.zeros((nb, sc.NT, 128, sc.IMW), np.float32)
    for t in range(sc.NT):
        r0 = sc.BS * t
        rows = min(128, sc.H - r0)
        out[:, t, :rows, : sc.H] = img[:, r0 : r0 + rows, :]
    return out


def _toeplitz_a(k: np.ndarray, kap: int) -> np.ndarray:
    """k: [n, kap, kap] -> wa[n, p, v, q] = k[n, p-q, v] for 0 <= p-q < kap."""
    p = np.arange(128)[:, None]
    q = np.arange(128)[None, :]
    d = p - q
    m = (d >= 0) & (d < kap)
    dc = np.where(m, d, 0)
    w = k[:, dc, :]  # [n, 128(p), 128(q), kap(v)]
    w = w.transpose(0, 1, 3, 2) * m[None, :, None, :]
    return np.ascontiguousarray(w.astype(np.float32))


# -------------------------------------------------------------- device build
_NC_CACHE: dict = {}

_MAX_WAITS = 1  # walrus setupSyncWait rejects multiple sem waits on some opcodes


def _split_excess_waits(nc, max_waits: int = _MAX_WAITS):
    """Hoist excess semaphore waits onto injected same-engine NoOps.

    Tile's vector-clock wait emission is not transitively minimal, so some
    instructions end up with 3+ sem waits, which walrus codegen rejects.
    Executing the same waits on an immediately preceding NoOp of the same
    engine is semantically identical (engines execute in program order).
    """
    n_id = 0
    for f in nc.m.functions:
        for bb in f.blocks:
            out = []
            changed = False
            for inst in bb.instructions:
                si = inst.sync_info
                waits = list(si.on_wait or []) if si is not None else []
                if len(waits) > max_waits:
                    keep = waits[-max_waits:]
                    excess = waits[:-max_waits]
                    for i in range(0, len(excess), max_waits):
                        n_id += 1
                        out.append(
                            mybir.InstNoOp(
                                name=f"waitnop-{n_id}",
                                engine=inst.engine,
                                bass_nofuse=True,
                                sync_info=mybir.SyncInfo(
                                    on_wait=excess[i : i + max_waits], on_update=[]
                                ),
                            )
                        )
                    inst.sync_info = mybir.SyncInfo(
                        on_wait=keep, on_update=list(si.on_update or [])
                    )
                    changed = True
                out.append(inst)
            if changed:
                bb.instructions = out


def _emit_sample(nc, sc: Scale, b: int, pools, params, consts):
    imgp, wtsp, cbufp, dbufp, obufp, ps_c, ps_d, ps_t, ps_o = pools
    img_p, wa_p, out_p = params[:3]
    ident, rta_t, rtb_t = consts

    img_t = imgp.tile([128, sc.NT, sc.IMW], MM_DT, tag=f"img{sc.l}", name=f"img{sc.l}_{b}")
    nc.sync.dma_start(
        out=img_t[:, :, :],
        in_=img_p[b].rearrange("t p c -> p t c"),
    )

    if sc.q8:
        img_pq, wa_pq = params[3], params[4]
        img8_t = imgp.tile(
            [128, sc.NT, sc.IMW], F8E4, tag=f"img{sc.l}q", name=f"img{sc.l}q_{b}"
        )
        nc.sync.dma_start(
            out=img8_t[:, :, :], in_=img_pq[b].rearrange("t p c -> p t c")
        )
        wa8_t = wtsp.tile([128, NQ8, 128], F8E4, tag=f"wa{sc.l}q", name=f"wa{sc.l}q_{b}")
        nc.sync.dma_start(out=wa8_t[:, :, :], in_=wa_pq[b])
        nex = sc.K - NQ8
    else:
        nex = sc.K

    wa_t = wtsp.tile([128, nex, 128], MM_DT, tag=f"wa{sc.l}", name=f"wa{sc.l}_{b}")
    nc.sync.dma_start(out=wa_t[:, :, :], in_=wa_p[b])

    # --- correlation + resize stage 1 (row contraction) ---
    d_ps = ps_d.tile([128, sc.NPAD], F32, tag="d", name=f"d_ps{sc.l}_{b}")
    for t in range(sc.NT):
        c_ps = ps_c.tile([128, sc.NPAD], F32, tag="c", name=f"c_ps{sc.l}_{b}_{t}")
        if sc.q8:
            # fp8 DoubleRow: pairs (v, v+1) share one PE pass; the rhs view
            # [128, 2, NPAD] overlaps the two column-shifted image windows.
            full8 = img8_t[:, :, :]
            for v in range(0, NQ8, 2):
                rhs = AP(
                    full8.tensor,
                    full8.offset + t * sc.IMW + v,
                    [list(full8.ap[0]), [1, 2], [1, sc.NPAD]],
                )
                nc.tensor.matmul(
                    c_ps[:, :],
                    wa8_t[:, v : v + 2, :],
                    rhs,
                    start=(v == 0),
                    stop=False,
                    perf_mode=DOUBLE_ROW,
                )
        for j in range(nex):
            v = sc.K - nex + j
            nc.tensor.matmul(
                c_ps[:, :],
                wa_t[:, j, :],
                img_t[:, t, v : v + sc.NPAD],
                start=(v == 0),
                stop=(v == sc.K - 1),
            )
        c_t = cbufp.tile([128, sc.NPAD], MM_DT, tag=f"c{sc.l}", name=f"c{sc.l}_{b}_{t}")
        # PSUM->SBUF casts alternate DVE/ACT to unload Vector (Pool can't read PSUM)
        ceng = (nc.vector.tensor_copy, nc.scalar.copy)[t % 2]
        ceng(c_t[:, :], c_ps[:, :])
        nc.tensor.matmul(
            d_ps[:, :],
            rta_t[:, t, :],
            c_t[:, :],
            start=(t == 0),
            stop=(t == sc.NT - 1),
        )

    dw = sc.NJC * 128  # D buffer width: full transpose chunks, f32-padded tail
    cw = min(sc.NPAD, dw)
    d_t = dbufp.tile([128, dw], F32, tag=f"d{sc.l}", name=f"d{sc.l}_{b}")
    ceng = (nc.vector.tensor_copy, nc.scalar.copy)[(b + 1) % 2]
    ceng(d_t[:, 0:cw], d_ps[:, 0:cw])
    if dw > sc.NPAD:
        nc.vector.memset(d_t[:, sc.NPAD : dw], 0.0)

    # --- transpose D, then resize stage 2 (col contraction) ---
    dt_t = dbufp.tile([128, sc.NJC, 128], MM_DT, tag=f"dt{sc.l}", name=f"dt{sc.l}_{b}")
    for jc in range(sc.NJC):
        tr_ps = ps_t.tile([128, 128], F32, tag="tr", name=f"tr_ps{sc.l}_{b}_{jc}")
        nc.tensor.transpose(tr_ps[:, :], d_t[:, 128 * jc : 128 * jc + 128], ident)
        ceng = (nc.vector.tensor_copy, nc.scalar.copy)[jc % 2]
        ceng(dt_t[:, jc, :], tr_ps[:, :])

    o_ps = ps_o.tile([128, 128], F32, tag="o", name=f"o_ps{sc.l}_{b}")
    for jc in range(sc.NJC):
        nc.tensor.matmul(
            o_ps[:, :],
            rtb_t[:, jc, :],
            dt_t[:, jc, :],
            start=(jc == 0),
            stop=(jc == sc.NJC - 1),
        )
    o_t = obufp.tile([128, 128], F32, tag="o", name=f"o{sc.l}_{b}")
    ceng = (nc.vector.tensor_copy, nc.scalar.copy)[b % 2]
    ceng(o_t[:, :], o_ps[:, :])
    nc.sync.dma_start(out=out_p[b], in_=o_t[:, :])


def _build_nc(split_waits: bool = True):
    key = ("nc", split_waits)
    if key in _NC_CACHE:
        return _NC_CACHE[key]

    nc = bass.Bass()
    img_p, wa_p, rta_p, rtb_p, out_p = [], [], [], [], []
    imgq_p, waq_p = {}, {}
    for sc in SCALES:
        img_p.append(
            nc.declare_dram_parameter(
                f"img{sc.l}", [PER, sc.NT, 128, sc.IMW], MM_DT, isOutput=False
            )
        )
        nex = sc.K - NQ8 if sc.q8 else sc.K
        wa_p.append(
            nc.declare_dram_parameter(f"wa{sc.l}", [PER, 128, nex, 128], MM_DT, isOutput=False)
        )
        if sc.q8:
            imgq_p[sc.l] = nc.declare_dram_parameter(
                f"img{sc.l}q", [PER, sc.NT, 128, sc.IMW], F8E4, isOutput=False
            )
            waq_p[sc.l] = nc.declare_dram_parameter(
                f"wa{sc.l}q", [PER, 128, NQ8, 128], F8E4, isOutput=False
            )
        rta_p.append(
            nc.declare_dram_parameter(f"rta{sc.l}", [128, sc.NT, 128], MM_DT, isOutput=False)
        )
        rtb_p.append(
            nc.declare_dram_parameter(f"rtb{sc.l}", [128, sc.NJC, 128], MM_DT, isOutput=False)
        )
        out_p.append(
            nc.declare_dram_parameter(f"out{sc.l}", [PER, 128, 128], F32, isOutput=True)
        )

    with TileContext(nc) as tc:
        with (
            tc.tile_pool(name="const", bufs=1) as constp,
            tc.tile_pool(name="img", bufs=2) as imgp,
            tc.tile_pool(name="wts", bufs=2) as wtsp,
            tc.tile_pool(name="cbuf", bufs=2) as cbufp,
            tc.tile_pool(name="dbuf", bufs=2) as dbufp,
            tc.tile_pool(name="obuf", bufs=2) as obufp,
            tc.tile_pool(name="ps_c", bufs=2, space="PSUM") as ps_c,
            tc.tile_pool(name="ps_d", bufs=2, space="PSUM") as ps_d,
            tc.tile_pool(name="ps_t", bufs=2, space="PSUM") as ps_t,
            tc.tile_pool(name="ps_o", bufs=2, space="PSUM") as ps_o,
        ):
            ident = constp.tile([128, 128], F32, name="ident")
            make_identity(nc, ident)
            rta_ts, rtb_ts = [], []
            for sc in SCALES:
                rta_t = constp.tile([128, sc.NT, 128], MM_DT, name=f"rta{sc.l}_t")
                nc.sync.dma_start(out=rta_t[:, :, :], in_=rta_p[sc.l][:, :, :])
                rta_ts.append(rta_t)
                rtb_t = constp.tile([128, sc.NJC, 128], MM_DT, name=f"rtb{sc.l}_t")
                nc.sync.dma_start(out=rtb_t[:, :, :], in_=rtb_p[sc.l][:, :, :])
                rtb_ts.append(rtb_t)

            pools = (imgp, wtsp, cbufp, dbufp, obufp, ps_c, ps_d, ps_t, ps_o)
            for b in range(PER):
                for sc in SCALES:
                    params = (img_p[sc.l], wa_p[sc.l], out_p[sc.l])
                    if sc.q8:
                        params = params + (imgq_p[sc.l], waq_p[sc.l])
                    _emit_sample(
                        nc,
                        sc,
                        b,
                        pools,
                        params,
                        (ident, rta_ts[sc.l], rtb_ts[sc.l]),
                    )

    if split_waits:
        _split_excess_waits(nc)
    _NC_CACHE[key] = nc
    return nc


# --------------------------------------------------------------------- entry
def kernel(s1, s2, s3, t1, t2, t3):
    global last_exec_time_ns, last_mean_exec_time_ns

    host_dt = ml_dtypes.bfloat16 if MM_DT == BF16 else np.float32
    imgs, imgs_q = [], {}
    for s, sc in zip((s1, s2, s3), SCALES):
        a = np.asarray(s, np.float32)[..., 0]
        tiled = _tile_image(a, sc)
        imgs.append(np.ascontiguousarray(tiled.astype(host_dt)))
        if sc.q8:
            imgs_q[sc.l] = np.ascontiguousarray(tiled.astype(E4))
    tmps = [np.asarray(t, np.float32)[..., 0] for t in (t1, t2, t3)]
    # flip both spatial dims (reference flips the template before the conv)
    ks = [np.ascontiguousarray(t[:, ::-1, ::-1]) for t in tmps]

    was, was_q = [], {}
    for sc in SCALES:
        wa = _toeplitz_a(ks[sc.l], sc.K)  # [n, 128, K, 128] f32
        if sc.q8:
            was_q[sc.l] = np.ascontiguousarray(wa[:, :, :NQ8, :].astype(E4))
            was.append(np.ascontiguousarray(wa[:, :, NQ8:, :].astype(host_dt)))
        else:
            was.append(np.ascontiguousarray(wa.astype(host_dt)))
    rtas = [_rta_chunks(sc).astype(host_dt) for sc in SCALES]
    rtbs = [_rtb_chunks(sc).astype(host_dt) for sc in SCALES]

    nc = _build_nc()

    in_maps = []
    for c in range(NCORES):
        sl = slice(PER * c, PER * (c + 1))
        m = {}
        for sc in SCALES:
            m[f"img{sc.l}"] = imgs[sc.l][sl]
            m[f"wa{sc.l}"] = was[sc.l][sl]
            m[f"rta{sc.l}"] = rtas[sc.l]
            m[f"rtb{sc.l}"] = rtbs[sc.l]
            if sc.q8:
                m[f"img{sc.l}q"] = imgs_q[sc.l][sl]
                m[f"wa{sc.l}q"] = was_q[sc.l][sl]
        in_maps.append(m)

    trace = bool(os.environ.get("KERNEL_TRACE"))
    tmpdir = os.environ.get("KERNEL_TRACE_DIR") or None
    if tmpdir:
        os.makedirs(tmpdir, exist_ok=True)
    res = run_bass_kernel_spmd(
        nc, in_maps, list(range(NCORES)), trace=trace, tmpdir=tmpdir
    )
    last_exec_time_ns = res.exec_time_ns
    last_mean_exec_time_ns = res.mean_exec_time_ns

    out = np.empty((B_TOTAL, OUT_HW, OUT_HW, 3), np.float32)
    for c in range(NCORES):
        sl = slice(PER * c, PER * (c + 1))
        for sc in SCALES:
            # device output is [b, x, y]; reference wants [b, y, x]
            out[sl, :, :, sc.l] = res.results[c][f"out{sc.l}"].swapaxes(1, 2)
    return out



# revision 16
# speedup vs baseline: 1.3928x; 1.0727x over previous
"""Trainium2 Bass kernel for nn_CrossCorrelation_38405597561347.

Reference computation (per sample b, scale l):
  C_l = valid_corr2d(s_l[b], flip2(t_l[b]))          # [L, L], L = H - k + 1
  out[b, :, :, l] = bilinear_resize(C_l, 128, 128)   # jax antialiased triangle
Output: [64, 128, 128, 3] float32.

Mapping to hardware (per core, 8 samples — pure batch data parallelism):
  * The row-resize is FUSED into the correlation weights (host-side):
       D[y, j] = (R C)[y, j] = sum_v sum_r G_v[r, y] s[r, j + v]
    with G_v[r, y] = sum_u R[y, r - u] k[u, v] dense [H, 128] matrices.
    Contraction over image rows r runs in H/128 dense 128-row chunks; the
    template-column shift v is a free-dim AP offset on the image.
  * Scale 0 splits v hybrid: 12 of 16 v's run as fp8e4 DoubleRow pairs
    (2x PE rate; pairs (v, v+1) share one PE pass via an overlapping
    [128, 2, NPAD] rhs view), 4 v's stay exact in bf16 to keep the global
    rel err ~1.9e-2 < 2e-2 (validated bit-faithfully on host).
  * Stage 2 (column resize) = PE-transpose of D then contraction with the
    (constant) resize matrix R. Output is [x, y] (transposed); host fixes
    layout during the final gather.
"""

import os
import sys
from dataclasses import dataclass

import ml_dtypes
import numpy as np

for _p in ("/opt/trn_rl_repo",):
    if _p not in sys.path and os.path.isdir(_p):
        sys.path.insert(0, _p)

import concourse.bass as bass
import concourse.mybir as mybir
from concourse.bass import AP
from concourse.bass_utils import run_bass_kernel_spmd
from concourse.masks import make_identity
from concourse.tile import TileContext

DOUBLE_ROW = mybir.MatmulPerfMode.DoubleRow
NQ8 = 12  # scale-0 template columns computed in fp8 (rest exact in bf16)

NCORES = 8
B_TOTAL = 64
PER = B_TOTAL // NCORES  # 8 samples per core
OUT_HW = 128

F32 = mybir.dt.float32
BF16 = mybir.dt.bfloat16
F8E4 = mybir.dt.float8e4
MM_DT = BF16
E4 = ml_dtypes.float8_e4m3
HOST_BF = ml_dtypes.bfloat16


@dataclass(frozen=True)
class Scale:
    l: int    # scale index
    H: int    # input image height/width
    K: int    # template size
    L: int    # valid correlation output size = H - K + 1
    NC: int   # dense contraction row-chunks (= H / 128)
    NPAD: int  # padded correlation free dim (>= L)
    IMW: int  # per-chunk SBUF width (>= NPAD + K - 1, cols zero-padded)
    NJC: int  # number of 128-col chunks fed to transpose/stage2
    q8: bool  # hybrid fp8e4 DoubleRow correlation


SCALES = (
    Scale(l=0, H=512, K=16, L=497, NC=4, NPAD=497, IMW=512, NJC=4, q8=True),
    Scale(l=1, H=256, K=8, L=249, NC=2, NPAD=256, IMW=264, NJC=2, q8=False),
    Scale(l=2, H=128, K=4, L=125, NC=1, NPAD=128, IMW=136, NJC=1, q8=False),
)

last_exec_time_ns = None
last_mean_exec_time_ns = None


# ----------------------------------------------------------------- host math
def _resize_mat(lin: int, lout: int = OUT_HW) -> np.ndarray:
    """R[y, i]: jax.image.resize bilinear (antialias=True) weight matrix."""
    inv_scale = lin / lout
    kernel_scale = max(inv_scale, 1.0)
    sample_f = (np.arange(lout, dtype=np.float64) + 0.5) * inv_scale - 0.5
    x = np.abs(sample_f[None, :] - np.arange(lin, dtype=np.float64)[:, None])
    w = np.maximum(0.0, 1.0 - x / kernel_scale)  # [lin, lout]
    w = w / w.sum(axis=0, keepdims=True)
    return np.ascontiguousarray(w.T.astype(np.float32))  # [lout, lin]


def _rtb_chunks(sc: Scale) -> np.ndarray:
    """Stage-2 R^T chunks over 128-aligned D^T tiles.

    [128, NJC, 128]: rtb[p, n, y] = R[y, 128n + p], zero beyond L."""
    r = _resize_mat(sc.L)  # [128, L]
    rt = np.zeros((128, sc.NJC, OUT_HW), np.float32)
    for n in range(sc.NJC):
        rows = min(128, sc.L - 128 * n)
        if rows > 0:
            rt[:rows, n, :] = r[:, 128 * n : 128 * n + rows].T
    return rt


def _chunk_image(img: np.ndarray, sc: Scale) -> np.ndarray:
    """[B, H, W] -> [B, NC, 128, IMW] dense row chunks, cols zero-padded."""
    nb = img.shape[0]
    out = np.zeros((nb, sc.NC, 128, sc.IMW), np.float32)
    out[:, :, :, : sc.H] = img.reshape(nb, sc.NC, 128, sc.H)
    return out


def _g_mats(ks: np.ndarray, sc: Scale) -> np.ndarray:
    """Row-resize-fused correlation weights.

    ks: [n, K, K] flipped templates ->
    G [n, NC, 128, K, 128]: G[b, q, p, v, y] = sum_u R[y, 128q + p - u] k[b, u, v]
    """
    n = ks.shape[0]
    r = _resize_mat(sc.L)  # [128, L]
    rsh = np.zeros((sc.K, sc.H, OUT_HW), np.float32)
    for u in range(sc.K):
        rsh[u, u : u + sc.L, :] = r.T
    kt = ks.transpose(0, 2, 1).reshape(n * sc.K, sc.K).astype(np.float32)
    g = (kt @ rsh.reshape(sc.K, sc.H * OUT_HW)).reshape(n, sc.K, sc.H, OUT_HW)
    # [n, v, (q p), y] -> [n, q, p, v, y]
    g = g.reshape(n, sc.K, sc.NC, 128, OUT_HW).transpose(0, 2, 3, 1, 4)
    return np.ascontiguousarray(g)


# -------------------------------------------------------------- device build
_NC_CACHE: dict = {}

_MAX_WAITS = 1  # walrus setupSyncWait rejects multiple sem waits on some opcodes


def _split_excess_waits(nc, max_waits: int = _MAX_WAITS):
    """Hoist excess semaphore waits onto injected same-engine NoOps.

    Tile's vector-clock wait emission is not transitively minimal, so some
    instructions end up with 3+ sem waits, which walrus codegen rejects.
    Executing the same waits on an immediately preceding NoOp of the same
    engine is semantically identical (engines execute in program order).
    """
    n_id = 0
    for f in nc.m.functions:
        for bb in f.blocks:
            out = []
            changed = False
            for inst in bb.instructions:
                si = inst.sync_info
                waits = list(si.on_wait or []) if si is not None else []
                if len(waits) > max_waits:
                    keep = waits[-max_waits:]
                    excess = waits[:-max_waits]
                    for i in range(0, len(excess), max_waits):
                        n_id += 1
                        out.append(
                            mybir.InstNoOp(
                                name=f"waitnop-{n_id}",
                                engine=inst.engine,
                                bass_nofuse=True,
                                sync_info=mybir.SyncInfo(
                                    on_wait=excess[i : i + max_waits], on_update=[]
                                ),
                            )
                        )
                    inst.sync_info = mybir.SyncInfo(
                        on_wait=keep, on_update=list(si.on_update or [])
                    )
                    changed = True
                out.append(inst)
            if changed:
                bb.instructions = out


def _emit_sample(nc, sc: Scale, b: int, pools, params, consts):
    imgp, wtsp, dbufp, obufp, ps_d, ps_t, ps_o = pools
    img_p, g_p, out_p = params[:3]
    ident, rtb_t = consts

    img_t = imgp.tile([128, sc.NC, sc.IMW], MM_DT, tag=f"img{sc.l}", name=f"img{sc.l}_{b}")
    nc.sync.dma_start(
        out=img_t[:, :, :],
        in_=img_p[b].rearrange("q p c -> p q c"),
    )

    if sc.q8:
        img_pq, g_pq = params[3], params[4]
        img8_t = imgp.tile(
            [128, sc.NC, sc.IMW], F8E4, tag=f"img{sc.l}q", name=f"img{sc.l}q_{b}"
        )
        nc.sync.dma_start(
            out=img8_t[:, :, :], in_=img_pq[b].rearrange("q p c -> p q c")
        )
        g8_t = wtsp.tile(
            [128, sc.NC, NQ8, 128], F8E4, tag=f"g{sc.l}q", name=f"g{sc.l}q_{b}"
        )
        nc.sync.dma_start(
            out=g8_t[:, :, :, :], in_=g_pq[b].rearrange("q p v y -> p q v y")
        )
        nex = sc.K - NQ8
    else:
        nex = sc.K

    g_t = wtsp.tile([128, sc.NC, nex, 128], MM_DT, tag=f"g{sc.l}", name=f"g{sc.l}_{b}")
    nc.sync.dma_start(out=g_t[:, :, :, :], in_=g_p[b].rearrange("q p v y -> p q v y"))

    # --- fused correlation + row resize: D = sum_{q,v} G_v[q]^T @ S_v[q] ---
    d_ps = ps_d.tile([128, sc.NPAD], F32, tag="d", name=f"d_ps{sc.l}_{b}")
    for q in range(sc.NC):
        if sc.q8:
            # fp8 DoubleRow: pairs (v, v+1) share one PE pass; the rhs view
            # [128, 2, NPAD] overlaps the two column-shifted image windows.
            full8 = img8_t[:, :, :]
            for v in range(0, NQ8, 2):
                rhs = AP(
                    full8.tensor,
                    full8.offset + q * sc.IMW + v,
                    [list(full8.ap[0]), [1, 2], [1, sc.NPAD]],
                )
                nc.tensor.matmul(
                    d_ps[:, :],
                    g8_t[:, q, v : v + 2, :],
                    rhs,
                    start=(q == 0 and v == 0),
                    stop=False,
                    perf_mode=DOUBLE_ROW,
                )
        for j in range(nex):
            v = sc.K - nex + j
            nc.tensor.matmul(
                d_ps[:, :],
                g_t[:, q, j, :],
                img_t[:, q, v : v + sc.NPAD],
                start=(q == 0 and v == 0),
                stop=(q == sc.NC - 1 and j == nex - 1),
            )

    dw = sc.NJC * 128  # D buffer width: full transpose chunks, f32-padded tail
    cw = min(sc.NPAD, dw)
    d_t = dbufp.tile([128, dw], F32, tag=f"d{sc.l}", name=f"d{sc.l}_{b}")
    ceng = (nc.vector.tensor_copy, nc.scalar.copy)[b % 2]
    ceng(d_t[:, 0:cw], d_ps[:, 0:cw])
    if dw > sc.NPAD:
        nc.vector.memset(d_t[:, sc.NPAD : dw], 0.0)

    # --- transpose D, then resize stage 2 (col contraction) ---
    dt_t = dbufp.tile([128, sc.NJC, 128], MM_DT, tag=f"dt{sc.l}", name=f"dt{sc.l}_{b}")
    for jc in range(sc.NJC):
        tr_ps = ps_t.tile([128, 128], F32, tag="tr", name=f"tr_ps{sc.l}_{b}_{jc}")
        nc.tensor.transpose(tr_ps[:, :], d_t[:, 128 * jc : 128 * jc + 128], ident)
        ceng = (nc.vector.tensor_copy, nc.scalar.copy)[jc % 2]
        ceng(dt_t[:, jc, :], tr_ps[:, :])

    o_ps = ps_o.tile([128, 128], F32, tag="o", name=f"o_ps{sc.l}_{b}")
    for jc in range(sc.NJC):
        nc.tensor.matmul(
            o_ps[:, :],
            rtb_t[:, jc, :],
            dt_t[:, jc, :],
            start=(jc == 0),
            stop=(jc == sc.NJC - 1),
        )
    o_t = obufp.tile([128, 128], F32, tag="o", name=f"o{sc.l}_{b}")
    ceng = (nc.vector.tensor_copy, nc.scalar.copy)[b % 2]
    ceng(o_t[:, :], o_ps[:, :])
    nc.sync.dma_start(out=out_p[b], in_=o_t[:, :])


def _build_nc(split_waits: bool = True):
    key = ("nc", split_waits)
    if key in _NC_CACHE:
        return _NC_CACHE[key]

    nc = bass.Bass()
    img_p, g_p, rtb_p, out_p = [], [], [], []
    imgq_p, gq_p = {}, {}
    for sc in SCALES:
        img_p.append(
            nc.declare_dram_parameter(
                f"img{sc.l}", [PER, sc.NC, 128, sc.IMW], MM_DT, isOutput=False
            )
        )
        nex = sc.K - NQ8 if sc.q8 else sc.K
        g_p.append(
            nc.declare_dram_parameter(
                f"g{sc.l}", [PER, sc.NC, 128, nex, 128], MM_DT, isOutput=False
            )
        )
        if sc.q8:
            imgq_p[sc.l] = nc.declare_dram_parameter(
                f"img{sc.l}q", [PER, sc.NC, 128, sc.IMW], F8E4, isOutput=False
            )
            gq_p[sc.l] = nc.declare_dram_parameter(
                f"g{sc.l}q", [PER, sc.NC, 128, NQ8, 128], F8E4, isOutput=False
            )
        rtb_p.append(
            nc.declare_dram_parameter(f"rtb{sc.l}", [128, sc.NJC, 128], MM_DT, isOutput=False)
        )
        out_p.append(
            nc.declare_dram_parameter(f"out{sc.l}", [PER, 128, 128], F32, isOutput=True)
        )

    with TileContext(nc) as tc:
        with (
            tc.tile_pool(name="const", bufs=1) as constp,
            tc.tile_pool(name="img", bufs=2) as imgp,
            tc.tile_pool(name="wts", bufs=2) as wtsp,
            tc.tile_pool(name="dbuf", bufs=2) as dbufp,
            tc.tile_pool(name="obuf", bufs=2) as obufp,
            tc.tile_pool(name="ps_d", bufs=3, space="PSUM") as ps_d,
            tc.tile_pool(name="ps_t", bufs=2, space="PSUM") as ps_t,
            tc.tile_pool(name="ps_o", bufs=2, space="PSUM") as ps_o,
        ):
            ident = constp.tile([128, 128], F32, name="ident")
            make_identity(nc, ident)
            rtb_ts = []
            for sc in SCALES:
                rtb_t = constp.tile([128, sc.NJC, 128], MM_DT, name=f"rtb{sc.l}_t")
                nc.sync.dma_start(out=rtb_t[:, :, :], in_=rtb_p[sc.l][:, :, :])
                rtb_ts.append(rtb_t)

            pools = (imgp, wtsp, dbufp, obufp, ps_d, ps_t, ps_o)
            for b in range(PER):
                for sc in SCALES:
                    params = (img_p[sc.l], g_p[sc.l], out_p[sc.l])
                    if sc.q8:
                        params = params + (imgq_p[sc.l], gq_p[sc.l])
                    _emit_sample(
                        nc,
                        sc,
                        b,
                        pools,
                        params,
                        (ident, rtb_ts[sc.l]),
                    )

    if split_waits:
        _split_excess_waits(nc)
    _NC_CACHE[key] = nc
    return nc


# --------------------------------------------------------------------- entry
def kernel(s1, s2, s3, t1, t2, t3):
    global last_exec_time_ns, last_mean_exec_time_ns

    imgs, imgs_q = [], {}
    for s, sc in zip((s1, s2, s3), SCALES):
        a = np.asarray(s, np.float32)[..., 0]
        chunks = _chunk_image(a, sc)
        imgs.append(np.ascontiguousarray(chunks.astype(HOST_BF)))
        if sc.q8:
            imgs_q[sc.l] = np.ascontiguousarray(chunks.astype(E4))
    tmps = [np.asarray(t, np.float32)[..., 0] for t in (t1, t2, t3)]
    # flip both spatial dims (reference flips the template before the conv)
    ks = [np.ascontiguousarray(t[:, ::-1, ::-1]) for t in tmps]

    gs, gs_q = [], {}
    for sc in SCALES:
        g = _g_mats(ks[sc.l], sc)  # [n, NC, 128, K, 128] f32
        if sc.q8:
            gs_q[sc.l] = np.ascontiguousarray(g[..., :NQ8, :].astype(E4))
            gs.append(np.ascontiguousarray(g[..., NQ8:, :].astype(HOST_BF)))
        else:
            gs.append(np.ascontiguousarray(g.astype(HOST_BF)))
    rtbs = [_rtb_chunks(sc).astype(HOST_BF) for sc in SCALES]

    nc = _build_nc()

    in_maps = []
    for c in range(NCORES):
        sl = slice(PER * c, PER * (c + 1))
        m = {}
        for sc in SCALES:
            m[f"img{sc.l}"] = imgs[sc.l][sl]
            m[f"g{sc.l}"] = gs[sc.l][sl]
            m[f"rtb{sc.l}"] = rtbs[sc.l]
            if sc.q8:
                m[f"img{sc.l}q"] = imgs_q[sc.l][sl]
                m[f"g{sc.l}q"] = gs_q[sc.l][sl]
        in_maps.append(m)

    trace = bool(os.environ.get("KERNEL_TRACE"))
    tmpdir = os.environ.get("KERNEL_TRACE_DIR") or None
    if tmpdir:
        os.makedirs(tmpdir, exist_ok=True)
    res = run_bass_kernel_spmd(
        nc, in_maps, list(range(NCORES)), trace=trace, tmpdir=tmpdir
    )
    last_exec_time_ns = res.exec_time_ns
    last_mean_exec_time_ns = res.mean_exec_time_ns

    out = np.empty((B_TOTAL, OUT_HW, OUT_HW, 3), np.float32)
    for c in range(NCORES):
        sl = slice(PER * c, PER * (c + 1))
        for sc in SCALES:
            # device output is [b, x, y]; reference wants [b, y, x]
            out[sl, :, :, sc.l] = res.results[c][f"out{sc.l}"].swapaxes(1, 2)
    return out
